# revision 1
# baseline (speedup 1.0000x reference)
"""Trainium2 Bass kernel for nn_ConsciousnessMonitor (histogram_binning).

kernel(**inputs) takes FULL unsharded numpy inputs, returns the full (9,)
float32 output. Shards state_history along time across 8 NeuronCores:
masked means via PE matmul while streaming, min/max + joint-histogram MI
with two small AllReduces, differentiation branch replicated per core.

Self-contained: shapes/sharding hardcoded; reads no sibling files.
"""
import numpy as np

import concourse.bacc as bacc
import concourse.tile as tile
import concourse.mybir as mybir
from concourse.bass_utils import run_bass_kernel_spmd
from concourse.masks import make_identity

F32 = mybir.dt.float32
I32 = mybir.dt.int32
AX = mybir.AxisListType
OP = mybir.AluOpType
ACT = mybir.ActivationFunctionType

N_CORES = 8
T, D = 32768, 2048
TL = T // N_CORES          # 4096 time steps per core
NB = 10                    # histogram bins per axis
NPAIR = 4                  # partitions (mask pairs)
J = 2 * NPAIR              # 8 masked-mean columns
NTC = TL // 512            # 8 accumulator groups (512 t each)
NDC = D // 128             # 16 contraction chunks
NCH = TL // 128            # 32 binning chunks of 128 t
MEM = 100
SN = 10

# accumulator tcn -> (bank b, quadrant q): tcn = 3*b + q, q in {0,1,2}
ACC_MAP = [(tcn // 3, tcn % 3) for tcn in range(NTC)]

_CACHE = {}
LAST_RESULTS = None


def _build(debug=False, variant="main"):
    sim1 = variant.startswith("sim1")
    nc = bacc.Bacc("TRN2", target_bir_lowering=False, debug=False,
                   num_devices=1 if sim1 else N_CORES)
    ht = nc.dram_tensor("ht", [D, TL], F32, kind="ExternalInput").ap()
    mmat = nc.dram_tensor("mmat", [D, J], F32, kind="ExternalInput").ap()
    invc = nc.dram_tensor("invc", [128, 1], F32, kind="ExternalInput").ap()
    memt = nc.dram_tensor("memt", [D, MEM], F32, kind="ExternalInput").ap()
    sampt = nc.dram_tensor("sampt", [D, SN], F32, kind="ExternalInput").ap()
    out = nc.dram_tensor("out", [9], F32, kind="ExternalOutput").ap()
    if debug:
        dbg_st = nc.dram_tensor("dbg_st", [J, 512], F32, kind="ExternalOutput").ap()
        dbg_gmm = nc.dram_tensor("dbg_gmm", [J, 2], F32, kind="ExternalOutput").ap()
        dbg_rmat = nc.dram_tensor("dbg_rmat", [J + 1, J], F32, kind="ExternalOutput").ap()
        dbg_bin = nc.dram_tensor("dbg_bin", [128, 16], I32, kind="ExternalOutput").ap()
        dbg_gj = nc.dram_tensor("dbg_gj", [NB, NPAIR * NB], F32, kind="ExternalOutput").ap()
        dbg_mm83 = nc.dram_tensor("dbg_mm83", [J, 9], F32, kind="ExternalOutput").ap()

    rg = [list(range(N_CORES))]

    with tile.TileContext(nc) as tc:
        with tc.tile_pool(name="consts", bufs=1) as consts, \
             tc.tile_pool(name="sb", bufs=1) as sb, \
             tc.tile_pool(name="htp", bufs=2) as htp, \
             tc.tile_pool(name="psA", bufs=3, space="PSUM") as psA_pool, \
             tc.tile_pool(name="psJ", bufs=2, space="PSUM") as psJ_pool, \
             tc.tile_pool(name="misc", bufs=3, space="PSUM") as misc, \
             tc.tile_pool(name="dram", bufs=1, space="DRAM") as dram:

            # ---- constants / small inputs ----
            ident10 = consts.tile([NB, NB], F32, tag="id10")
            make_identity(nc, ident10[:])
            ones128 = consts.tile([128, 1], F32, tag="o128")
            nc.gpsimd.memset(ones128[:], 1.0)
            ones10 = consts.tile([NB, 1], F32, tag="o10")
            nc.gpsimd.memset(ones10[:], 1.0)
            ones1_10 = consts.tile([1, NB], F32, tag="o110")
            nc.gpsimd.memset(ones1_10[:], 1.0)
            ones8x8 = consts.tile([J, J], F32, tag="o88")
            nc.gpsimd.memset(ones8x8[:], 1.0)

            m_sb = consts.tile([128, NDC * J], F32, tag="msb")
            nc.gpsimd.dma_start(
                out=m_sb[:].rearrange("p (k j) -> p k j", j=J),
                in_=mmat.rearrange("(k p) j -> p k j", p=128))
            invc_sb = consts.tile([128, 1], F32, tag="invc")
            nc.gpsimd.dma_start(out=invc_sb[:], in_=invc[:])
            mem_sb = consts.tile([128, NDC * MEM], F32, tag="memsb")
            nc.gpsimd.dma_start(
                out=mem_sb[:].rearrange("p (k f) -> p k f", f=MEM),
                in_=memt.rearrange("(k p) f -> p k f", p=128))
            samp_sb = consts.tile([128, NDC * SN], F32, tag="sampsb")
            nc.gpsimd.dma_start(
                out=samp_sb[:].rearrange("p (k f) -> p k f", f=SN),
                in_=sampt.rearrange("(k p) f -> p k f", p=128))

            # ---- differentiation branch: Gram + row norms (early PE) ----
            psG = misc.tile([SN, SN], F32, tag="m")
            for k in range(NDC):
                nc.tensor.matmul(psG[:], samp_sb[:, k * SN:(k + 1) * SN],
                                 samp_sb[:, k * SN:(k + 1) * SN],
                                 start=(k == 0), stop=(k == NDC - 1))
            sqs = sb.tile([128, NDC * SN], F32, tag="sqs")
            nc.vector.tensor_tensor(sqs[:], samp_sb[:], samp_sb[:], OP.mult)
            psr = misc.tile([SN, 1], F32, tag="m")
            for k in range(NDC):
                nc.tensor.matmul(psr[:], sqs[:, k * SN:(k + 1) * SN],
                                 ones128[:], start=(k == 0),
                                 stop=(k == NDC - 1))
            g_sb = sb.tile([SN, SN], F32, tag="gsb")
            nc.scalar.copy(g_sb[:], psG[:])
            r_sb = sb.tile([SN, 1], F32, tag="rsb")
            nc.scalar.copy(r_sb[:], psr[:])

            # variance branch (DVE; overlaps stream)
            mem3 = mem_sb[:].rearrange("p (k f) -> p k f", f=MEM)
            mean16 = sb.tile([128, NDC], F32, tag="mean16")
            nc.vector.tensor_reduce(mean16[:], mem3, AX.X, OP.add)
            nc.vector.tensor_scalar(mean16[:], mean16[:], 1.0 / MEM, None,
                                    OP.mult)
            cent = sb.tile([128, NDC * MEM], F32, tag="cent")
            nc.vector.tensor_tensor(
                cent[:].rearrange("p (k f) -> p k f", f=MEM), mem3,
                mean16[:, :, None].broadcast_to([128, NDC, MEM]), OP.subtract)
            nc.vector.tensor_tensor(cent[:], cent[:], cent[:], OP.mult)
            var16 = sb.tile([128, NDC], F32, tag="var16")
            nc.vector.tensor_reduce(
                var16[:], cent[:].rearrange("p (k f) -> p k f", f=MEM),
                AX.X, OP.add)
            nc.vector.tensor_scalar(var16[:], var16[:], 1.0 / (MEM - 1), None,
                                    OP.mult)
            redv = sb.tile([128, 1], F32, tag="redv")
            nc.vector.tensor_reduce(redv[:], var16[:], AX.X, OP.add)
            v2 = sb.tile([128, NDC], F32, tag="v2")
            nc.vector.tensor_tensor(v2[:], var16[:], var16[:], OP.mult)
            redv2 = sb.tile([128, 1], F32, tag="redv2")
            nc.vector.tensor_reduce(redv2[:], v2[:], AX.X, OP.add)
            pstv = misc.tile([1, 1], F32, tag="m")
            nc.tensor.matmul(pstv[:], redv[:], ones128[:], start=True,
                             stop=True)
            tv_sb = sb.tile([1, 1], F32, tag="tvsb")
            nc.scalar.copy(tv_sb[:], pstv[:])
            pss2 = misc.tile([1, 1], F32, tag="m")
            nc.tensor.matmul(pss2[:], redv2[:], ones128[:], start=True,
                             stop=True)
            s2_sb = sb.tile([1, 1], F32, tag="s2sb")
            nc.scalar.copy(s2_sb[:], pss2[:])

            tvsq = sb.tile([1, 1], F32, tag="tvsq")
            nc.vector.tensor_tensor(tvsq[:], tv_sb[:], tv_sb[:], OP.mult)
            dden = sb.tile([1, 1], F32, tag="dden")
            nc.vector.scalar_tensor_tensor(dden[:], tvsq[:], 1e-6, s2_sb[:],
                                           OP.mult, OP.add)
            rdden = sb.tile([1, 1], F32, tag="rdden")
            nc.vector.reciprocal(rdden[:], dden[:])
            eff_sb = sb.tile([1, 1], F32, tag="effsb")
            nc.vector.tensor_tensor(eff_sb[:], tvsq[:], rdden[:], OP.mult)

            # cdist tail: d2 = r_i + r_j - 2G
            rrow_ps = misc.tile([1, SN], F32, tag="m")
            nc.tensor.transpose(rrow_ps[:], r_sb[:], ident10[:])
            rrow = sb.tile([1, SN], F32, tag="rrow")
            nc.scalar.copy(rrow[:], rrow_ps[:])
            rB = misc.tile([SN, SN], F32, tag="m")
            nc.tensor.matmul(rB[:], ones1_10[:], rrow[:], start=True,
                             stop=True)
            d2 = sb.tile([SN, SN], F32, tag="d2")
            nc.vector.scalar_tensor_tensor(d2[:], g_sb[:], -2.0, rB[:],
                                           OP.mult, OP.add)
            nc.vector.tensor_scalar(d2[:], d2[:], r_sb[:], 0.0, OP.add,
                                    OP.max)
            dst = sb.tile([SN, SN], F32, tag="dst")
            nc.scalar.activation(dst[:], d2[:], ACT.Sqrt)
            dsum = sb.tile([SN, 1], F32, tag="dsum")
            nc.vector.tensor_reduce(dsum[:], dst[:], AX.X, OP.add)
            psD = misc.tile([1, 1], F32, tag="m")
            nc.tensor.matmul(psD[:], dsum[:], ones10[:], start=True, stop=True)
            avg_sb = sb.tile([1, 1], F32, tag="avgsb")
            nc.vector.tensor_scalar(avg_sb[:], psD[:],
                                    float(1.0 / (SN * (SN - 1) + 1e-6)), None,
                                    OP.mult)
            sqtv = sb.tile([1, 1], F32, tag="sqtv")
            nc.scalar.activation(sqtv[:], tv_sb[:], ACT.Sqrt)
            diff_sb = sb.tile([1, 1], F32, tag="diffsb")
            nc.vector.tensor_tensor(diff_sb[:], sqtv[:], avg_sb[:], OP.mult)
            tanhd = sb.tile([1, 1], F32, tag="tanhd")
            nc.scalar.activation(tanhd[:], diff_sb[:], ACT.Tanh)

            # ---- stage A: stream HT, S.T = M.T @ HT into 3 packed banks ----
            psA = [psA_pool.tile([128, 512], F32, tag="sacc", name=f"psA{i}")
                   for i in range(3)]
            if variant in ("bigdma4", "bigdma8", "sim1big"):
                ng = 8 if variant == "bigdma8" else 4
                per = NDC // ng          # d-chunks per DMA group
                ht3 = ht.rearrange("(c p) t -> p c t", p=128)
                for g in range(ng):
                    htt = htp.tile([128, per * TL], F32, tag="htt",
                                   name="htt")
                    nc.sync.dma_start(
                        out=htt[:].rearrange("p (c t) -> p c t", t=TL),
                        in_=ht3[:, g * per:(g + 1) * per, :])
                    for ci in range(per):
                        dk = g * per + ci
                        for tcn in range(NTC):
                            b, q = ACC_MAP[tcn]
                            nc.tensor.matmul(
                                psA[b][32 * q:32 * q + J, :],
                                m_sb[:, dk * J:(dk + 1) * J],
                                htt[:, ci * TL + tcn * 512:
                                    ci * TL + (tcn + 1) * 512],
                                start=(dk == 0), stop=(dk == NDC - 1))
            else:
                sched = {"main": list(range(NDC)), "ndc1": [0],
                         "stream3": list(range(NDC)) * 3, "sim1": list(range(NDC)),
                         "multiq": list(range(NDC))}[variant]
                qengs = ([nc.sync, nc.scalar, nc.vector, nc.gpsimd]
                         if variant == "multiq" else [nc.sync])
                for i, dk in enumerate(sched):
                    htt = htp.tile([128, TL], F32, tag="htt", name="htt")
                    qengs[i % len(qengs)].dma_start(
                        out=htt[:], in_=ht[dk * 128:(dk + 1) * 128, :])
                    for tcn in range(NTC):
                        b, q = ACC_MAP[tcn]
                        nc.tensor.matmul(psA[b][32 * q:32 * q + J, :],
                                         m_sb[:, dk * J:(dk + 1) * J],
                                         htt[:, tcn * 512:(tcn + 1) * 512],
                                         start=(i == 0),
                                         stop=(i == len(sched) - 1))

            # ---- stage B: scale to SBUF (lane-aligned), min/max, AllReduce --
            # stS layout: [128, 3*512]; group (b,q): rows 32q..32q+7 hold
            # S.T rows for tcn=3b+q at cols b*512.., row 32q+8 = ones.
            stS = sb.tile([128, 3 * 512], F32, tag="sts")
            ones1536 = sb.tile([1, 3 * 512], F32, tag="ones1536")
            nc.vector.memset(ones1536[:], 1.0)
            for q in range(3):
                nc.sync.dma_start(out=stS[32 * q + J:32 * q + J + 1, :],
                                  in_=ones1536[:])
            mxb = sb.tile([128, 3], F32, tag="mxb")
            mnb = sb.tile([128, 3], F32, tag="mnb")
            nc.gpsimd.memset(mxb[:], -3.0e38)
            nc.gpsimd.memset(mnb[:], 3.0e38)
            for tcn in range(NTC):
                b, q = ACC_MAP[tcn]
                seg = stS[32 * q:32 * q + J, b * 512:(b + 1) * 512]
                nc.scalar.mul(seg, psA[b][32 * q:32 * q + J, :],
                              invc_sb[32 * q:32 * q + J, :])
                nc.vector.tensor_reduce(mxb[32 * q:32 * q + J, b:b + 1], seg,
                                        AX.X, OP.max)
                nc.vector.tensor_reduce(mnb[32 * q:32 * q + J, b:b + 1], seg,
                                        AX.X, OP.min)
            # gather lanes {32q+j} -> [8, 3] via SBUF->SBUF DMA remap
            mx83 = sb.tile([J, 9], F32, tag="mx83")
            mn83 = sb.tile([J, 9], F32, tag="mn83")
            for q in range(3):
                nc.sync.dma_start(out=mx83[:, 3 * q:3 * q + 3],
                                  in_=mxb[32 * q:32 * q + J, :])
                nc.sync.dma_start(out=mn83[:, 3 * q:3 * q + 3],
                                  in_=mnb[32 * q:32 * q + J, :])
            minmax = sb.tile([J, 2], F32, tag="minmax")
            nc.vector.tensor_reduce(minmax[:, 0:1], mx83[:], AX.X, OP.max)
            tmn = sb.tile([J, 1], F32, tag="tmn")
            nc.vector.tensor_reduce(tmn[:], mn83[:], AX.X, OP.min)
            nc.vector.tensor_scalar(minmax[:, 1:2], tmn[:], -1.0, None,
                                    OP.mult)
            cbA = dram.tile([J, 2], F32, tag="cba")
            cbB = dram.tile([J, 2], F32, tag="cbb")
            nc.gpsimd.dma_start(out=cbA[:], in_=minmax[:])
            if sim1:
                nc.gpsimd.dma_start(out=cbB[:], in_=cbA[:])
            else:
                nc.gpsimd.collective_compute("AllReduce", OP.max,
                                             replica_groups=rg,
                                             ins=[cbA.opt()],
                                             outs=[cbB.opt()])
            gmm = sb.tile([J, 2], F32, tag="gmm")
            nc.gpsimd.dma_start(out=gmm[:], in_=cbB[:])

            # s1 = 10/(max-min+1e-6); b1 = -min*s1 - 0.5 (RNE cast -> floor)
            gmn = sb.tile([J, 1], F32, tag="gmn")
            nc.vector.tensor_scalar(gmn[:], gmm[:, 1:2], -1.0, None, OP.mult)
            dden2 = sb.tile([J, 1], F32, tag="dden2")
            nc.vector.tensor_tensor(dden2[:], gmm[:, 0:1], gmn[:], OP.subtract)
            nc.vector.tensor_scalar(dden2[:], dden2[:], 1e-6, None, OP.add)
            rdd = sb.tile([J, 1], F32, tag="rdd")
            nc.vector.reciprocal(rdd[:], dden2[:])
            s1 = sb.tile([J, 1], F32, tag="s1")
            nc.vector.tensor_scalar(s1[:], rdd[:], 10.0, None, OP.mult)
            b1 = sb.tile([J, 1], F32, tag="b1")
            nc.vector.tensor_tensor(b1[:], gmn[:], s1[:], OP.mult)
            nc.vector.tensor_scalar(b1[:], b1[:], -1.0, -0.5, OP.mult, OP.add)

            # R [9,8] replicated at partition bases 0/32/64:
            # rows 32q..32q+7 diag(s1), row 32q+8 = b1 row
            s1b = sb.tile([J, J], F32, tag="s1b")
            nc.vector.tensor_scalar(s1b[:], ones8x8[:], s1[:], None, OP.mult)
            rmat = sb.tile([128, J], F32, tag="rmat")
            nc.gpsimd.memset(rmat[:], 0.0)
            nc.gpsimd.affine_select(out=rmat[0:J, :], in_=s1b[:],
                                    compare_op=OP.is_equal, fill=0.0, base=0,
                                    pattern=[[-1, J]], channel_multiplier=1)
            nc.sync.dma_start(out=rmat[J:J + 1, 0:J], in_=b1[:])
            nc.sync.dma_start(out=rmat[32:32 + J + 1, :], in_=rmat[0:J + 1, :])
            nc.sync.dma_start(out=rmat[64:64 + J + 1, :], in_=rmat[0:J + 1, :])

            # ---- stage C: affine+transpose via PE, bin, one-hot, joints ----
            psC = misc.tile([128, NCH * J], F32, tag="m")
            for tcn in range(NTC):
                b, q = ACC_MAP[tcn]
                for c in range(4):
                    gc = tcn * 4 + c
                    nc.tensor.matmul(
                        psC[:, gc * J:(gc + 1) * J],
                        stS[32 * q:32 * q + J + 1,
                            b * 512 + c * 128:b * 512 + (c + 1) * 128],
                        rmat[32 * q:32 * q + J + 1, :],
                        start=True, stop=True)
            binint = sb.tile([128, NCH * J], I32, tag="binint")
            nc.vector.tensor_copy(binint[:], psC[:])
            nc.vector.tensor_scalar(binint[:], binint[:], 0, NB - 1, OP.max,
                                    OP.min)
            ohsb = sb.tile([128, NCH * J * NB], F32, tag="ohsb")
            oh3 = ohsb[:].rearrange("p (c b) -> p c b", b=NB)
            for b in range(NB):
                nc.vector.tensor_scalar(oh3[:, :, b], binint[:], b, None,
                                        OP.is_equal)
            # joint histograms: psJt1 packs pairs 0..2 at bases 0/32/64
            psJt1 = psJ_pool.tile([128, NB], F32, tag="pj", name="psJt1")
            psJt2 = psJ_pool.tile([NB, NB], F32, tag="pj", name="psJt2")
            for p in range(NPAIR):
                outap = (psJt2[:] if p == 3
                         else psJt1[32 * p:32 * p + NB, :])
                for c in range(NCH):
                    xa = (c * J + 2 * p) * NB
                    ya = (c * J + 2 * p + 1) * NB
                    nc.tensor.matmul(outap, ohsb[:, xa:xa + NB],
                                     ohsb[:, ya:ya + NB], start=(c == 0),
                                     stop=(c == NCH - 1))
            jm1 = sb.tile([128, NB], F32, tag="jm1")
            jm2 = sb.tile([NB, NB], F32, tag="jm2")
            for p in range(3):
                nc.scalar.copy(jm1[32 * p:32 * p + NB, :],
                               psJt1[32 * p:32 * p + NB, :])
            nc.scalar.copy(jm2[:], psJt2[:])
            cbj = dram.tile([NPAIR, NB * NB], F32, tag="cbj")
            cbj2 = dram.tile([NPAIR, NB * NB], F32, tag="cbj2")
            for p in range(3):
                nc.gpsimd.dma_start(
                    out=cbj[p:p + 1, :],
                    in_=jm1[32 * p:32 * p + NB, :])
            nc.gpsimd.dma_start(out=cbj[3:4, :], in_=jm2[:])
            if sim1:
                nc.gpsimd.dma_start(out=cbj2[:], in_=cbj[:])
            else:
                nc.gpsimd.collective_compute("AllReduce", OP.add,
                                             replica_groups=rg,
                                             ins=[cbj.opt()],
                                             outs=[cbj2.opt()])
            gj = sb.tile([NB, NPAIR * NB], F32, tag="gj")
            nc.gpsimd.dma_start(
                out=gj[:].rearrange("a (p b) -> a p b", b=NB),
                in_=cbj2[:].rearrange("p (a b) -> a p b", a=NB))

            # ---- stage D: MI per pair ----
            mirow = sb.tile([1, NPAIR], F32, tag="mirow")
            for p in range(NPAIR):
                gjp = gj[:, p * NB:(p + 1) * NB]
                rowsum = sb.tile([NB, 1], F32, tag="rowsum", name="rowsum")
                nc.vector.tensor_reduce(rowsum[:], gjp, AX.X, OP.add)
                colps = misc.tile([NB, 1], F32, tag="m", name="colps")
                nc.tensor.matmul(colps[:], gjp, ones10[:], start=True,
                                 stop=True)
                totps = misc.tile([1, 1], F32, tag="m", name="totps")
                nc.tensor.matmul(totps[:], rowsum[:], ones10[:], start=True,
                                 stop=True)
                tot = sb.tile([1, 1], F32, tag="tot", name="tot")
                nc.vector.tensor_scalar(tot[:], totps[:], 1e-10, None, OP.add)
                tinv = sb.tile([1, 1], F32, tag="tinv", name="tinv")
                nc.vector.reciprocal(tinv[:], tot[:])
                t10ps = misc.tile([NB, 1], F32, tag="m", name="t10ps")
                nc.tensor.matmul(t10ps[:], ones1_10[:], tinv[:], start=True,
                                 stop=True)
                t10 = sb.tile([NB, 1], F32, tag="t10", name="t10")
                nc.scalar.copy(t10[:], t10ps[:])
                jn = sb.tile([NB, NB], F32, tag="jn", name="jn")
                nc.vector.tensor_scalar(jn[:], gjp, t10[:], None, OP.mult)
                px = sb.tile([NB, 1], F32, tag="px", name="px")
                nc.vector.tensor_scalar(px[:], rowsum[:], t10[:], None,
                                        OP.mult)
                py = sb.tile([NB, 1], F32, tag="py", name="py")
                nc.vector.tensor_scalar(py[:], colps[:], t10[:], None,
                                        OP.mult)
                pyr_ps = misc.tile([1, NB], F32, tag="m", name="pyr_ps")
                nc.tensor.transpose(pyr_ps[:], py[:], ident10[:])
                pyr = sb.tile([1, NB], F32, tag="pyr", name="pyr")
                nc.scalar.copy(pyr[:], pyr_ps[:])
                pyB = misc.tile([NB, NB], F32, tag="m", name="pyB")
                nc.tensor.matmul(pyB[:], ones1_10[:], pyr[:], start=True,
                                 stop=True)
                outer = sb.tile([NB, NB], F32, tag="outer", name="outer")
                nc.vector.tensor_scalar(outer[:], pyB[:], px[:], None,
                                        OP.mult)
                num = sb.tile([NB, NB], F32, tag="num", name="num")
                nc.vector.tensor_scalar(num[:], jn[:], 1e-10, None, OP.add)
                nc.vector.tensor_scalar(outer[:], outer[:], 1e-10, None,
                                        OP.add)
                rout = sb.tile([NB, NB], F32, tag="rout", name="rout")
                nc.vector.reciprocal(rout[:], outer[:])
                nc.vector.tensor_tensor(num[:], num[:], rout[:], OP.mult)
                lg = sb.tile([NB, NB], F32, tag="lg", name="lg")
                nc.scalar.activation(lg[:], num[:], ACT.Ln)
                nc.vector.tensor_tensor(lg[:], jn[:], lg[:], OP.mult)
                ms = sb.tile([NB, 1], F32, tag="ms", name="ms")
                nc.vector.tensor_reduce(ms[:], lg[:], AX.X, OP.add)
                mips = misc.tile([1, 1], F32, tag="m", name="mips")
                nc.tensor.matmul(mips[:], ms[:], ones10[:], start=True,
                                 stop=True)
                nc.vector.tensor_scalar(mirow[:, p:p + 1], mips[:], 0.0, None,
                                        OP.max)

            integ = sb.tile([1, 1], F32, tag="integ")
            nc.vector.tensor_reduce(integ[:], mirow[:], AX.X, OP.min)
            consc = sb.tile([1, 1], F32, tag="consc")
            nc.vector.tensor_tensor(consc[:], integ[:], tanhd[:], OP.add)

            outrow = sb.tile([1, 9], F32, tag="outrow")
            nc.vector.tensor_copy(outrow[:, 0:1], consc[:])
            nc.vector.tensor_copy(outrow[:, 1:2], diff_sb[:])
            nc.vector.tensor_copy(outrow[:, 2:3], eff_sb[:])
            nc.vector.tensor_copy(outrow[:, 3:4], tv_sb[:])
            nc.vector.tensor_copy(outrow[:, 4:5], integ[:])
            nc.vector.tensor_copy(outrow[:, 5:9], mirow[:])
            nc.sync.dma_start(out=out[:], in_=outrow[:])
            if debug:
                nc.sync.dma_start(out=dbg_st[:], in_=stS[0:J, 0:512])
                nc.sync.dma_start(out=dbg_gmm[:], in_=gmm[:])
                nc.sync.dma_start(out=dbg_rmat[:], in_=rmat[0:J + 1, :])
                nc.sync.dma_start(out=dbg_bin[:], in_=binint[:, 0:16])
                nc.sync.dma_start(out=dbg_gj[:], in_=gj[:])
                nc.sync.dma_start(out=dbg_mm83[:], in_=mx83[:])

    nc.compile()
    return nc


def _build_variant(name):
    return _build(variant=name)


def _get_nc(debug=False):
    key = ("ncd" if debug else "nc")
    if key not in _CACHE:
        _CACHE[key] = _build(debug)
    return _CACHE[key]


def kernel(state, state_memory, state_history, partitions, sample_idx,
           trace=False, debug=False):
    global LAST_RESULTS
    state = np.asarray(state, np.float32)
    state_memory = np.asarray(state_memory, np.float32)
    state_history = np.asarray(state_history, np.float32)
    partitions = np.asarray(partitions)
    sample_idx = np.asarray(sample_idx)

    mmat = np.empty((D, J), np.float32)
    invc8 = np.empty((J,), np.float32)
    pf = partitions.astype(np.float32)
    for p in range(NPAIR):
        mmat[:, 2 * p] = pf[p]
        mmat[:, 2 * p + 1] = np.float32(1.0) - pf[p]
        invc8[2 * p] = np.float32(1.0) / pf[p].sum(dtype=np.float32)
        invc8[2 * p + 1] = np.float32(1.0) / (np.float32(1.0) - pf[p]).sum(
            dtype=np.float32)
    invc = np.zeros((128, 1), np.float32)
    for q in range(3):
        invc[32 * q:32 * q + J, 0] = invc8
    memory = np.concatenate([state, state_memory[state.shape[0]:]], axis=0)
    memt = np.ascontiguousarray(memory.T)
    sampt = np.ascontiguousarray(memory[sample_idx].T)

    in_maps = []
    for c in range(N_CORES):
        htc = np.ascontiguousarray(state_history[c * TL:(c + 1) * TL, :].T)
        in_maps.append({"ht": htc, "mmat": mmat, "invc": invc,
                        "memt": memt, "sampt": sampt})

    nc = _get_nc(debug)
    res = run_bass_kernel_spmd(nc, in_maps, list(range(N_CORES)),
                               trace=trace)
    LAST_RESULTS = res
    return np.asarray(res.results[0]["out"], np.float32)



# revision 9
# speedup vs baseline: 1.6691x; 1.6691x over previous
"""Trainium2 Bass kernel for nn_ConsciousnessMonitor (histogram_binning).

kernel(**inputs) takes FULL unsharded numpy inputs, returns the full (9,)
float32 output. Shards state_history along time across 8 NeuronCores:
masked means via bf16 PE matmul while streaming (f32->bf16 cast in the
DMA), raw-sum min/max + joint-histogram MI with two small AllReduces,
differentiation branch replicated per core.

Self-contained: shapes/sharding hardcoded; reads no sibling files.
"""
import numpy as np
import ml_dtypes

import concourse.bacc as bacc
import concourse.tile as tile
import concourse.mybir as mybir
from concourse.bass_utils import run_bass_kernel_spmd
from concourse.masks import make_identity

F32 = mybir.dt.float32
BF16 = mybir.dt.bfloat16
I32 = mybir.dt.int32
AX = mybir.AxisListType
OP = mybir.AluOpType
ACT = mybir.ActivationFunctionType

N_CORES = 8
T, D = 32768, 2048
TL = T // N_CORES          # 4096 time steps per core
NB = 10                    # histogram bins per axis
NPAIR = 4                  # partitions (mask pairs)
J = 2 * NPAIR              # 8 masked-mean columns
NTC = TL // 512            # 8 accumulator groups (512 t each)
NDC = D // 128             # 16 contraction chunks
NCH = TL // 128            # 32 binning chunks of 128 t
MEM = 100
SN = 10

# accumulator tcn -> (bank b, quadrant q): tcn = 3*b + q, q in {0,1,2}
ACC_MAP = [(tcn // 3, tcn % 3) for tcn in range(NTC)]

_CACHE = {}
LAST_RESULTS = None


def _build(debug=False, variant="main"):
    sim1 = variant.startswith("sim1")
    nc = bacc.Bacc("TRN2", target_bir_lowering=False, debug=False,
                   num_devices=1 if sim1 else N_CORES)
    ht = nc.dram_tensor("ht", [D, TL], F32, kind="ExternalInput").ap()
    mmat = nc.dram_tensor("mmat", [128, NDC * J], BF16,
                          kind="ExternalInput").ap()
    invc = nc.dram_tensor("invc", [128, 1], F32, kind="ExternalInput").ap()
    memt = nc.dram_tensor("memt", [128, NDC * MEM], F32,
                          kind="ExternalInput").ap()
    sampt = nc.dram_tensor("sampt", [128, NDC * SN], F32,
                           kind="ExternalInput").ap()
    sel = nc.dram_tensor("sel", [128, 3 * J], F32, kind="ExternalInput").ap()
    idrep = nc.dram_tensor("idrep", [128, J], F32, kind="ExternalInput").ap()
    bd440 = nc.dram_tensor("bd440", [NPAIR, NPAIR * NB], F32,
                           kind="ExternalInput").ap()
    bd404 = nc.dram_tensor("bd404", [NPAIR * NB, NPAIR], F32,
                           kind="ExternalInput").ap()
    out = nc.dram_tensor("out", [9], F32, kind="ExternalOutput").ap()

    rg = [list(range(N_CORES))]

    with tile.TileContext(nc) as tc:
        with tc.tile_pool(name="consts", bufs=1) as consts, \
             tc.tile_pool(name="sb", bufs=1) as sb, \
             tc.tile_pool(name="htp", bufs=2) as htp, \
             tc.tile_pool(name="psA", bufs=3, space="PSUM") as psA_pool, \
             tc.tile_pool(name="psJ", bufs=2, space="PSUM") as psJ_pool, \
             tc.tile_pool(name="misc", bufs=3, space="PSUM") as misc, \
             tc.tile_pool(name="dram", bufs=1, space="DRAM") as dram:

            # ---- on-chip constants ----
            ident10 = consts.tile([NB, NB], F32, tag="id10")
            make_identity(nc, ident10[:])
            ones128 = consts.tile([128, 1], F32, tag="o128")
            nc.gpsimd.memset(ones128[:], 1.0)
            ones10 = consts.tile([NB, 1], F32, tag="o10")
            nc.gpsimd.memset(ones10[:], 1.0)
            ones1_10 = consts.tile([1, NB], F32, tag="o110")
            nc.gpsimd.memset(ones1_10[:], 1.0)


            # ---- small input loads (host pre-swizzled; HWDGE) ----
            m_sb = consts.tile([128, NDC * J], BF16, tag="msb")
            nc.sync.dma_start(out=m_sb[:], in_=mmat[:])
            invc_sb = consts.tile([128, 1], F32, tag="invc")
            nc.sync.dma_start(out=invc_sb[:], in_=invc[:])
            samp_sb = consts.tile([128, NDC * SN], F32, tag="sampsb")
            nc.sync.dma_start(out=samp_sb[:], in_=sampt[:])
            mem_sb = consts.tile([128, NDC * MEM], F32, tag="memsb")
            nc.sync.dma_start(out=mem_sb[:], in_=memt[:])
            sel_sb = consts.tile([128, 3 * J], F32, tag="selsb")
            nc.sync.dma_start(out=sel_sb[:], in_=sel[:])
            idrep_sb = consts.tile([128, J], F32, tag="idrepsb")
            nc.sync.dma_start(out=idrep_sb[:], in_=idrep[:])
            bd440_sb = consts.tile([NPAIR, NPAIR * NB], F32, tag="bd440sb")
            nc.sync.dma_start(out=bd440_sb[:], in_=bd440[:])
            bd404_sb = consts.tile([NPAIR * NB, NPAIR], F32, tag="bd404sb")
            nc.sync.dma_start(out=bd404_sb[:], in_=bd404[:])

            # ---- stage A: stream HT (f32->bf16 cast DMA), S.T = M.T @ HT --
            psA = [psA_pool.tile([128, 512], F32, tag="sacc", name=f"psA{i}")
                   for i in range(3)]
            # clear stale PSUM rows (gather/reduce read all 128 lanes)
            for b in range(3):
                nc.vector.memset(psA[b][:], 0.0)
            ones1536 = sb.tile([1, 3 * 512], F32, tag="ones1536")
            nc.vector.memset(ones1536[:], 1.0)

            for dk in range(NDC):
                htt = htp.tile([128, TL], BF16, tag="htt", name="htt")
                nc.gpsimd.dma_start(out=htt[:],
                                    in_=ht[dk * 128:(dk + 1) * 128, :])
                for tcn in range(NTC):
                    b, q = ACC_MAP[tcn]
                    nc.tensor.matmul(psA[b][32 * q:32 * q + J, :],
                                     m_sb[:, dk * J:(dk + 1) * J],
                                     htt[:, tcn * 512:(tcn + 1) * 512],
                                     start=(dk == 0), stop=(dk == NDC - 1))

            # ---- differentiation branch (overlaps stream) ----
            psG = misc.tile([SN, SN], F32, tag="m", name="psG")
            for k in range(NDC):
                nc.tensor.matmul(psG[:], samp_sb[:, k * SN:(k + 1) * SN],
                                 samp_sb[:, k * SN:(k + 1) * SN],
                                 start=(k == 0), stop=(k == NDC - 1))
            sqs = sb.tile([128, NDC * SN], F32, tag="sqs")
            nc.vector.tensor_tensor(sqs[:], samp_sb[:], samp_sb[:], OP.mult)
            psr = misc.tile([SN, 1], F32, tag="m", name="psr")
            for k in range(NDC):
                nc.tensor.matmul(psr[:], sqs[:, k * SN:(k + 1) * SN],
                                 ones128[:], start=(k == 0),
                                 stop=(k == NDC - 1))
            g_sb = sb.tile([SN, SN], F32, tag="gsb")
            nc.scalar.copy(g_sb[:], psG[:])
            r_sb = sb.tile([SN, 1], F32, tag="rsb")
            nc.scalar.copy(r_sb[:], psr[:])

            # variance branch (DVE; overlaps stream)
            mem3 = mem_sb[:].rearrange("p (k f) -> p k f", f=MEM)
            mean16 = sb.tile([128, NDC], F32, tag="mean16")
            nc.vector.tensor_reduce(mean16[:], mem3, AX.X, OP.add)
            nc.vector.tensor_scalar(mean16[:], mean16[:], 1.0 / MEM, None,
                                    OP.mult)
            cent = sb.tile([128, NDC * MEM], F32, tag="cent")
            nc.vector.tensor_tensor(
                cent[:].rearrange("p (k f) -> p k f", f=MEM), mem3,
                mean16[:, :, None].broadcast_to([128, NDC, MEM]), OP.subtract)
            nc.vector.tensor_tensor(cent[:], cent[:], cent[:], OP.mult)
            var16 = sb.tile([128, NDC], F32, tag="var16")
            nc.vector.tensor_reduce(
                var16[:], cent[:].rearrange("p (k f) -> p k f", f=MEM),
                AX.X, OP.add)
            nc.vector.tensor_scalar(var16[:], var16[:], 1.0 / (MEM - 1), None,
                                    OP.mult)
            redv = sb.tile([128, 1], F32, tag="redv")
            nc.vector.tensor_reduce(redv[:], var16[:], AX.X, OP.add)
            v2 = sb.tile([128, NDC], F32, tag="v2")
            nc.vector.tensor_tensor(v2[:], var16[:], var16[:], OP.mult)
            redv2 = sb.tile([128, 1], F32, tag="redv2")
            nc.vector.tensor_reduce(redv2[:], v2[:], AX.X, OP.add)
            pstv = misc.tile([1, 1], F32, tag="m", name="pstv")
            nc.tensor.matmul(pstv[:], redv[:], ones128[:], start=True,
                             stop=True)
            tv_sb = sb.tile([1, 1], F32, tag="tvsb")
            nc.scalar.copy(tv_sb[:], pstv[:])
            pss2 = misc.tile([1, 1], F32, tag="m", name="pss2")
            nc.tensor.matmul(pss2[:], redv2[:], ones128[:], start=True,
                             stop=True)
            s2_sb = sb.tile([1, 1], F32, tag="s2sb")
            nc.scalar.copy(s2_sb[:], pss2[:])

            tvsq = sb.tile([1, 1], F32, tag="tvsq")
            nc.vector.tensor_tensor(tvsq[:], tv_sb[:], tv_sb[:], OP.mult)
            dden = sb.tile([1, 1], F32, tag="dden")
            nc.vector.scalar_tensor_tensor(dden[:], tvsq[:], 1e-6, s2_sb[:],
                                           OP.mult, OP.add)
            rdden = sb.tile([1, 1], F32, tag="rdden")
            nc.vector.reciprocal(rdden[:], dden[:])
            eff_sb = sb.tile([1, 1], F32, tag="effsb")
            nc.vector.tensor_tensor(eff_sb[:], tvsq[:], rdden[:], OP.mult)

            # cdist tail: d2 = r_i + r_j - 2G
            rrow_ps = misc.tile([1, SN], F32, tag="m", name="rrow_ps")
            nc.tensor.transpose(rrow_ps[:], r_sb[:], ident10[:])
            rrow = sb.tile([1, SN], F32, tag="rrow")
            nc.scalar.copy(rrow[:], rrow_ps[:])
            rB = misc.tile([SN, SN], F32, tag="m", name="rB")
            nc.tensor.matmul(rB[:], ones1_10[:], rrow[:], start=True,
                             stop=True)
            d2 = sb.tile([SN, SN], F32, tag="d2")
            nc.vector.scalar_tensor_tensor(d2[:], g_sb[:], -2.0, rB[:],
                                           OP.mult, OP.add)
            nc.vector.tensor_scalar(d2[:], d2[:], r_sb[:], 0.0, OP.add,
                                    OP.max)
            dst = sb.tile([SN, SN], F32, tag="dst")
            nc.scalar.activation(dst[:], d2[:], ACT.Sqrt)
            dsum = sb.tile([SN, 1], F32, tag="dsum")
            nc.vector.tensor_reduce(dsum[:], dst[:], AX.X, OP.add)
            psD = misc.tile([1, 1], F32, tag="m", name="psD")
            nc.tensor.matmul(psD[:], dsum[:], ones10[:], start=True, stop=True)
            avg_sb = sb.tile([1, 1], F32, tag="avgsb")
            nc.vector.tensor_scalar(avg_sb[:], psD[:],
                                    float(1.0 / (SN * (SN - 1) + 1e-6)), None,
                                    OP.mult)
            sqtv = sb.tile([1, 1], F32, tag="sqtv")
            nc.scalar.activation(sqtv[:], tv_sb[:], ACT.Sqrt)
            diff_sb = sb.tile([1, 1], F32, tag="diffsb")
            nc.vector.tensor_tensor(diff_sb[:], sqtv[:], avg_sb[:], OP.mult)
            tanhd = sb.tile([1, 1], F32, tag="tanhd")
            nc.scalar.activation(tanhd[:], diff_sb[:], ACT.Tanh)

            # ---- stage B: raw min/max per bank, PE lane-gather, AllReduce --
            stS = sb.tile([128, 3 * 512], F32, tag="sts")
            mm6 = sb.tile([128, 6], F32, tag="mm6")
            for b in range(3):
                nc.vector.tensor_reduce(mm6[:, b:b + 1], psA[b][:], AX.X,
                                        OP.max)
                nc.vector.tensor_reduce(mm6[:, 3 + b:4 + b], psA[b][:], AX.X,
                                        OP.min)
                nc.scalar.copy(stS[:, b * 512:(b + 1) * 512], psA[b][:])
            for q in range(3):
                nc.sync.dma_start(out=stS[32 * q + J:32 * q + J + 1, :],
                                  in_=ones1536[:])
            psMM = misc.tile([J, 18], F32, tag="m", name="psMM")
            for q in range(3):
                nc.tensor.matmul(psMM[:, q * 6:(q + 1) * 6],
                                 sel_sb[:, q * J:(q + 1) * J], mm6[:],
                                 start=True, stop=True)
            mm18 = sb.tile([J, 18], F32, tag="mm18")
            nc.scalar.copy(mm18[:], psMM[:])
            mmq = sb.tile([J, 6], F32, tag="mmq")
            nc.vector.tensor_tensor(mmq[:, 0:3], mm18[:, 0:3], mm18[:, 6:9],
                                    OP.max)
            nc.vector.tensor_tensor(mmq[:, 0:3], mmq[:, 0:3], mm18[:, 12:15],
                                    OP.max)
            nc.vector.tensor_tensor(mmq[:, 3:6], mm18[:, 3:6], mm18[:, 9:12],
                                    OP.min)
            nc.vector.tensor_tensor(mmq[:, 3:6], mmq[:, 3:6], mm18[:, 15:18],
                                    OP.min)
            minmax = sb.tile([J, 2], F32, tag="minmax")
            nc.vector.tensor_reduce(minmax[:, 0:1], mmq[:, 0:3], AX.X, OP.max)
            tmn = sb.tile([J, 1], F32, tag="tmn")
            nc.vector.tensor_reduce(tmn[:], mmq[:, 3:6], AX.X, OP.min)
            nc.vector.tensor_scalar(minmax[:, 1:2], tmn[:], -1.0, None,
                                    OP.mult)
            cbA = dram.tile([J, 2], F32, tag="cba")
            cbB = dram.tile([J, 2], F32, tag="cbb")
            nc.sync.dma_start(out=cbA[:], in_=minmax[:])
            if sim1:
                nc.sync.dma_start(out=cbB[:], in_=cbA[:])
            else:
                nc.gpsimd.collective_compute("AllReduce", OP.max,
                                             replica_groups=rg,
                                             ins=[cbA.opt()],
                                             outs=[cbB.opt()])
            # replicate global raw min/max to the three quadrant row groups
            gmm = sb.tile([128, 2], F32, tag="gmm")
            for q in range(3):
                nc.sync.dma_start(out=gmm[32 * q:32 * q + J, :], in_=cbB[:])

            # s1 = 10/((max-min)*invc + 1e-6); s1' = s1*invc;
            # b1 = -min*invc*s1 - 0.5 (RNE cast -> floor)  [raw-sum domain]
            gmx = gmm[:, 0:1]
            gmn = sb.tile([128, 1], F32, tag="gmn")
            nc.vector.tensor_scalar(gmn[:], gmm[:, 1:2], -1.0, None, OP.mult)
            dden2 = sb.tile([128, 1], F32, tag="dden2")
            nc.vector.tensor_tensor(dden2[:], gmx, gmn[:], OP.subtract)
            nc.vector.tensor_scalar(dden2[:], dden2[:], invc_sb[:], 1e-6,
                                    OP.mult, OP.add)
            rdd = sb.tile([128, 1], F32, tag="rdd")
            nc.vector.reciprocal(rdd[:], dden2[:])
            s1 = sb.tile([128, 1], F32, tag="s1")
            nc.vector.tensor_scalar(s1[:], rdd[:], 10.0, None, OP.mult)
            s1p = sb.tile([128, 1], F32, tag="s1p")
            nc.vector.tensor_tensor(s1p[:], s1[:], invc_sb[:], OP.mult)
            b1 = sb.tile([128, 1], F32, tag="b1")
            nc.vector.tensor_tensor(b1[:], gmn[:], s1p[:], OP.mult)
            nc.vector.tensor_scalar(b1[:], b1[:], -1.0, -0.5, OP.mult, OP.add)

            # rmat: diag(s1') at rows 32q+j, b1 row at 32q+8
            rmat = sb.tile([128, J], F32, tag="rmat")
            nc.vector.tensor_scalar(rmat[:], idrep_sb[:], s1p[:], None,
                                    OP.mult)
            b1t_ps = misc.tile([1, J], F32, tag="m", name="b1t_ps")
            nc.tensor.transpose(b1t_ps[:], b1[0:J, :], ident10[0:J, 0:J])
            b1row = sb.tile([1, J], F32, tag="b1row")
            nc.scalar.copy(b1row[:], b1t_ps[:])
            for q in range(3):
                nc.sync.dma_start(out=rmat[32 * q + J:32 * q + J + 1, :],
                                  in_=b1row[:])

            # ---- stage C: affine+transpose via PE, bin, one-hot, joints ----
            psC = misc.tile([128, NCH * J], F32, tag="m", name="psC")
            for tcn in range(NTC):
                b, q = ACC_MAP[tcn]
                for c in range(4):
                    gc = tcn * 4 + c
                    nc.tensor.matmul(
                        psC[:, gc * J:(gc + 1) * J],
                        stS[32 * q:32 * q + J + 1,
                            b * 512 + c * 128:b * 512 + c * 128 + 128],
                        rmat[32 * q:32 * q + J + 1, :],
                        start=True, stop=True)
            binint = sb.tile([128, NCH * J], I32, tag="binint")
            nc.vector.tensor_copy(binint[:], psC[:])
            nc.vector.tensor_scalar(binint[:], binint[:], 0, NB - 1, OP.max,
                                    OP.min)
            ohsb = sb.tile([128, NCH * J * NB], F32, tag="ohsb")
            oh3 = ohsb[:].rearrange("p (c b) -> p c b", b=NB)
            for b in range(NB):
                nc.vector.tensor_scalar(oh3[:, :, b], binint[:], b, None,
                                        OP.is_equal)
            # joint histograms: psJt1 packs pairs 0..2 at bases 0/32/64
            psJt1 = psJ_pool.tile([128, NB], F32, tag="pj", name="psJt1")
            psJt2 = psJ_pool.tile([NB, NB], F32, tag="pj", name="psJt2")
            jm1 = sb.tile([128, NB], F32, tag="jm1")
            jm2 = sb.tile([NB, NB], F32, tag="jm2")
            cbj = dram.tile([NPAIR, NB * NB], F32, tag="cbj")
            cbj2 = dram.tile([NPAIR, NB * NB], F32, tag="cbj2")
            for p in range(NPAIR):
                outap = (psJt2[:] if p == 3
                         else psJt1[32 * p:32 * p + NB, :])
                for c in range(NCH):
                    xa = (c * J + 2 * p) * NB
                    ya = (c * J + 2 * p + 1) * NB
                    nc.tensor.matmul(outap, ohsb[:, xa:xa + NB],
                                     ohsb[:, ya:ya + NB], start=(c == 0),
                                     stop=(c == NCH - 1))
                if p == 3:
                    nc.scalar.copy(jm2[:], psJt2[:])
                    nc.sync.dma_start(out=cbj[3:4, :], in_=jm2[:])
                else:
                    nc.scalar.copy(jm1[32 * p:32 * p + NB, :],
                                   psJt1[32 * p:32 * p + NB, :])
                    nc.sync.dma_start(out=cbj[p:p + 1, :],
                                      in_=jm1[32 * p:32 * p + NB, :])
            if sim1:
                nc.sync.dma_start(out=cbj2[:], in_=cbj[:])
            else:
                nc.gpsimd.collective_compute("AllReduce", OP.add,
                                             replica_groups=rg,
                                             ins=[cbj.opt()],
                                             outs=[cbj2.opt()])
            # gj4: pairs stacked along partitions, [40, 10]
            gj4 = sb.tile([NPAIR * NB, NB], F32, tag="gj4")
            nc.sync.dma_start(
                out=gj4[:],
                in_=cbj2[:].rearrange("p (a b) -> (p a) b", b=NB))

            # ---- stage D: MI for all 4 pairs at once ----
            rowsum = sb.tile([NPAIR * NB, 1], F32, tag="rowsum")
            nc.vector.tensor_reduce(rowsum[:], gj4[:], AX.X, OP.add)
            colps = misc.tile([NPAIR, NB], F32, tag="m", name="colps")
            nc.tensor.matmul(colps[:], bd404_sb[:], gj4[:], start=True,
                             stop=True)
            tot4 = sb.tile([NPAIR, 1], F32, tag="tot4")
            nc.vector.tensor_reduce(tot4[:], colps[:], AX.X, OP.add)
            nc.vector.tensor_scalar(tot4[:], tot4[:], 1e-10, None, OP.add)
            tinv = sb.tile([NPAIR, 1], F32, tag="tinv")
            nc.vector.reciprocal(tinv[:], tot4[:])
            t40_ps = misc.tile([NPAIR * NB, 1], F32, tag="m", name="t40_ps")
            nc.tensor.matmul(t40_ps[:], bd440_sb[:], tinv[:], start=True,
                             stop=True)
            t40 = sb.tile([NPAIR * NB, 1], F32, tag="t40")
            nc.scalar.copy(t40[:], t40_ps[:])
            pyn = sb.tile([NPAIR, NB], F32, tag="pyn")
            nc.vector.tensor_scalar(pyn[:], colps[:], tinv[:], None, OP.mult)
            pyB = misc.tile([NPAIR * NB, NB], F32, tag="m", name="pyB")
            nc.tensor.matmul(pyB[:], bd440_sb[:], pyn[:], start=True,
                             stop=True)
            px = sb.tile([NPAIR * NB, 1], F32, tag="px")
            nc.vector.tensor_tensor(px[:], rowsum[:], t40[:], OP.mult)
            jn = sb.tile([NPAIR * NB, NB], F32, tag="jn")
            nc.vector.tensor_scalar(jn[:], gj4[:], t40[:], None, OP.mult)
            num = sb.tile([NPAIR * NB, NB], F32, tag="num")
            nc.vector.tensor_scalar(num[:], jn[:], 1e-10, None, OP.add)
            outer = sb.tile([NPAIR * NB, NB], F32, tag="outer")
            nc.vector.tensor_scalar(outer[:], pyB[:], px[:], 1e-10, OP.mult,
                                    OP.add)
            rout = sb.tile([NPAIR * NB, NB], F32, tag="rout")
            nc.vector.reciprocal(rout[:], outer[:])
            nc.vector.tensor_tensor(num[:], num[:], rout[:], OP.mult)
            lg = sb.tile([NPAIR * NB, NB], F32, tag="lg")
            nc.scalar.activation(lg[:], num[:], ACT.Ln)
            nc.vector.tensor_tensor(lg[:], jn[:], lg[:], OP.mult)
            ms = sb.tile([NPAIR * NB, 1], F32, tag="ms")
            nc.vector.tensor_reduce(ms[:], lg[:], AX.X, OP.add)
            mi4_ps = misc.tile([NPAIR, 1], F32, tag="m", name="mi4_ps")
            nc.tensor.matmul(mi4_ps[:], bd404_sb[:], ms[:], start=True,
                             stop=True)
            mi4 = sb.tile([NPAIR, 1], F32, tag="mi4")
            nc.vector.tensor_scalar(mi4[:], mi4_ps[:], 0.0, None, OP.max)
            mit_ps = misc.tile([1, NPAIR], F32, tag="m", name="mit_ps")
            nc.tensor.transpose(mit_ps[:], mi4[:], ident10[0:NPAIR, 0:NPAIR])
            mirow = sb.tile([1, NPAIR], F32, tag="mirow")
            nc.scalar.copy(mirow[:], mit_ps[:])

            integ = sb.tile([1, 1], F32, tag="integ")
            nc.vector.tensor_reduce(integ[:], mirow[:], AX.X, OP.min)
            consc = sb.tile([1, 1], F32, tag="consc")
            nc.vector.tensor_tensor(consc[:], integ[:], tanhd[:], OP.add)

            outrow = sb.tile([1, 9], F32, tag="outrow")
            nc.vector.tensor_copy(outrow[:, 0:1], consc[:])
            nc.vector.tensor_copy(outrow[:, 1:2], diff_sb[:])
            nc.vector.tensor_copy(outrow[:, 2:3], eff_sb[:])
            nc.vector.tensor_copy(outrow[:, 3:4], tv_sb[:])
            nc.vector.tensor_copy(outrow[:, 4:5], integ[:])
            nc.vector.tensor_copy(outrow[:, 5:9], mirow[:])
            nc.sync.dma_start(out=out[:], in_=outrow[:])

    nc.compile()
    return nc


def _get_nc(debug=False):
    key = ("ncd" if debug else "nc")
    if key not in _CACHE:
        _CACHE[key] = _build(debug)
    return _CACHE[key]


def kernel(state, state_memory, state_history, partitions, sample_idx,
           trace=False, debug=False):
    global LAST_RESULTS
    state = np.asarray(state, np.float32)
    state_memory = np.asarray(state_memory, np.float32)
    state_history = np.asarray(state_history, np.float32)
    partitions = np.asarray(partitions)
    sample_idx = np.asarray(sample_idx)

    pf = partitions.astype(np.float32)
    mmat = np.empty((D, J), np.float32)
    invc8 = np.empty((J,), np.float32)
    for p in range(NPAIR):
        mmat[:, 2 * p] = pf[p]
        mmat[:, 2 * p + 1] = np.float32(1.0) - pf[p]
        invc8[2 * p] = np.float32(1.0) / pf[p].sum(dtype=np.float32)
        invc8[2 * p + 1] = np.float32(1.0) / (np.float32(1.0) - pf[p]).sum(
            dtype=np.float32)
    invc = np.zeros((128, 1), np.float32)
    for q in range(3):
        invc[32 * q:32 * q + J, 0] = invc8

    # SBUF-layout pre-swizzles: [p, k*F + f] = src[k*128 + p, f]
    def swz(src_dxf):
        f = src_dxf.shape[1]
        return np.ascontiguousarray(
            src_dxf.reshape(NDC, 128, f).transpose(1, 0, 2).reshape(
                128, NDC * f))

    mmatb = swz(mmat).astype(ml_dtypes.bfloat16)
    memory = np.concatenate([state, state_memory[state.shape[0]:]], axis=0)
    memt = swz(np.ascontiguousarray(memory.T))
    sampt = swz(np.ascontiguousarray(memory[sample_idx].T))

    sel = np.zeros((128, 3 * J), np.float32)
    idrep = np.zeros((128, J), np.float32)
    for q in range(3):
        for j in range(J):
            sel[32 * q + j, q * J + j] = 1.0
            idrep[32 * q + j, j] = 1.0
    bd440 = np.zeros((NPAIR, NPAIR * NB), np.float32)
    bd404 = np.zeros((NPAIR * NB, NPAIR), np.float32)
    for p in range(NPAIR):
        bd440[p, p * NB:(p + 1) * NB] = 1.0
        bd404[p * NB:(p + 1) * NB, p] = 1.0

    in_maps = []
    for c in range(N_CORES):
        htc = np.ascontiguousarray(state_history[c * TL:(c + 1) * TL, :].T)
        in_maps.append({"ht": htc, "mmat": mmatb, "invc": invc,
                        "memt": memt, "sampt": sampt, "sel": sel,
                        "idrep": idrep, "bd440": bd440, "bd404": bd404})

    nc = _get_nc(debug)
    res = run_bass_kernel_spmd(nc, in_maps, list(range(N_CORES)),
                               trace=trace)
    LAST_RESULTS = res
    return np.asarray(res.results[0]["out"], np.float32)
